# revision 48
# baseline (speedup 1.0000x reference)
"""Locally-connected conv (BioConvolution) Trainium2 kernel.

Problem: Z[n,p,o] = relu(sum_{ijc} patch[n,p,i,j,c] * filt[p,i,j,c,o] + bias[o])
  X: (32,128,128,32) f32, filters: (1024,4,4,32,32) f32, bias: (32,)
  out: (32,32,32,32) f32.   FH=FW=4 non-overlapping patches, P=1024.

Sharding: patch-parallel over P across 8 cores. Core k owns patches
[128k,128k+128) == image rows [16k,16k+16); no operand is reused anywhere,
so the problem is pure streaming and HBM/DMA-bound.

Shipped variant "bf16s" (~40 us NEFF exec; fp32 baseline was ~62 us):
  - Host casts both operands to bf16 (rel err 2.9e-3 vs the 2e-2 gate),
    halving input traffic to 8.4 MB/core; additionally, for the first 96
    of each core's 128 patches, the X operand of the last two K-chunks
    (q=2,3) rides as fp8-e4m3 (filters stay bf16), saving another 0.8 MB
    at a measured total rel err of 1.64e-2 — still under the gate.
    Output is stored bf16 and upcast on the host.
  - Host marshaling puts the contraction on SBUF partitions:
    xt[r, p, q, b] = X[b, 16k+4*pr+q, 4*pc+j, c] (r = j*32+c), filters
    matching; X/filters packed together per chunk so every HBM->SBUF DMA
    moves 128 partitions x multi-KB contiguous runs.
  - All input loads ride the sync engine's single HWDGE ring, issued
    upfront. Measured ring behavior that shaped the schedule: one ring
    sustains ~410-440 GB/s only with LARGE chunks (a chunk's 128
    descriptors are handed to the 16 DMA engines serially, so chunks
    much under ~2 MB underrun the engines); every chunk's completion
    semaphore reaches its target ~3 us after the data lands (one
    straggler engine notification), so the tail uses small chunks whose
    notify lags overlap; a tiny DMA at a ring head stalls that ring ~3 us
    (bias therefore rides the scalar ring, whose latency is harmless).
  - Per patch: 4 accumulating bf16 matmuls (K=128, M=32 fout, N=32
    batch) at 1 cycle/row (fp32r ran at 4 cycles/row at peak clock —
    switching dtypes also took the PE off the critical path). 8 patches
    pack side-by-side along the free axis of one PSUM bank [32, 8x32].
  - ScalarE applies bias+ReLU per PSUM group into bf16 staging; stores
    ride ScalarE's ring, bulk ones lagged behind the ACT stream and the
    final one small so the last ACT->store chain is short.
Remaining fixed overheads (~17 us): ~8.5 us engine boot + Tile preamble
before the first DMA packet, ~3 us tail notify lag, ~3.5 us Tile drain
barrier + semaphore resets, ~2 us last-chunk compute/store chain.

Session-2 findings (trace-verified; bf16s re-confirmed champion at
median 40.0 us over 8 warm runs, spread 39.7-40.9):
  - Exec decomposes as core-0 WORK (~33.0 us) + a fixed 6,626 ns
    runtime end-handshake measured identical on every core (it is NOT a
    wait-for-other-cores barrier: slow cores show the same gap).
  - WORK = 2.6 preamble-to-first-byte + 18.9 stream (16 engines @
    ~25.4 GB/s each, saturated; ~401 GB/s aggregate, above the 358
    nominal) + 3.1 engine-0 straggle + 2.7 PE/ACT tail + 1.6 store
    chain + ~2.0 drain barrier/sem-resets + 0.6 handshake entry.
  - The "notify lag" is NOT a fixed receipt delay: a chunk's completion
    sem fires ~0.4 us after the LAST ENGINE's share of its descriptors
    drains. The real gate is DMA engine 0, which alone carries the PE
    sequencer's 4x16 KB IRAM block refills (Q_XIV row-14 DMAs, ~4.3 us)
    and so finishes its input FIFO ~3 us after engines 1-15. Work
    conservation makes this unfixable by scheduling: desc i of every
    DMA maps to engine i%16, so engine 0 always gets >= 1/16 of every
    transfer and all refill bytes. Shrinking the PE program below 512
    matmuls is impossible (block-diag packing puts diagonal blocks on
    disjoint PSUM partitions; DVE/ACT are lane-local, so folding them
    back costs more than the refills).
  - Byte-saving restructures all MEASURED WORSE than bf16s despite
    -0.26 MB traffic: full-coverage fp8 q23-X as per-chunk DMAs
    (41.9 us), as one upfront mega-DMA (42.2), keeping bf16s shape with
    an early tail-fp8 DMA (41.0), packed single-DMA chunks (R1a), and a
    graduated (16,8,6,2) tail (40.1, tie). The 9-DMA bf16s stream shape
    is a local optimum; error budget at full coverage is 1.888e-2
    (deterministic, numpy-verified) vs the 2e-2 gate if ever needed.
  - Session-4: store_late (defer bulk stores past the input stream so
    their engine-time stops extending it) ties bf16s (40.44 vs 40.21
    interleaved medians, 4-4). Device has multi-minute slow phases at
    ~44.5 us that snap back to ~39.8 with no kernel change.
  - Session-3 (tail-latency attacks, all interleaved-A/B measured):
    the PE tail runs at the throttled 1.2 GHz clock (HAM flips cold on
    a ~2.4 us mid-stream I$-refill stall whose patch position MOVES run
    to run), so a fixed-site dummy-matmul warm-keeper (warm={84:10},
    ~40.7) can't catch it. Alternating PSUM evictions Scalar/DVE
    (act_alt, 40.3) ties bf16s; adding a (16,16,32,32) graduated head
    regresses (~41.1, +2 DMA boundaries). Machine-level drift of
    +/-1.5 us over minutes makes consecutive-batch A/B misleading —
    always interleave variant runs.
"""

import numpy as np
import ml_dtypes

N, H, W, C = 32, 128, 128, 32
FH = FW = 4
FOUT = 32
NCORES = 8
PL = 128          # patches per core
NQ = 4            # K-chunks per patch (512 / 128)
KR = 128          # contraction rows per chunk (SBUF partitions)
NG = PL // 4      # 4-patch groups per core

_CACHE = {}


def _build_module(bufs=6, out_splits=8, mm_dtype="float32"):
    from concourse import bacc, tile, mybir

    nc = bacc.Bacc("TRN2", target_bir_lowering=False, debug=False, enable_asserts=False)
    dt = mybir.dt.float32
    mdt = getattr(mybir.dt, mm_dtype)
    # xf packs data and filters: [..., 0:32] = batch cols, [..., 32:64] = fout
    xf = nc.dram_tensor("xf", [KR, PL, NQ, N + FOUT], mdt, kind="ExternalInput").ap()
    bt = nc.dram_tensor("bt", [KR, 1], dt, kind="ExternalInput").ap()
    out = nc.dram_tensor("out", [KR, NG, N], dt, kind="ExternalOutput").ap()

    # Graduated chunk sizes (in patches): small first chunks so the first
    # matmul isn't gated on a full-size load sharing bandwidth round-robin.
    sizes = [2, 2, 4]
    rest = PL - sum(sizes)
    sizes += [8] * (rest // 8)
    assert sum(sizes) == PL
    GSPLIT = NG // out_splits
    relu = mybir.ActivationFunctionType.Relu

    with tile.TileContext(nc) as tc:
        with (
            tc.tile_pool(name="xfpool", bufs=bufs) as xfpool,
            tc.tile_pool(name="psum", bufs=8, space="PSUM") as psum,
            tc.tile_pool(name="misc", bufs=1) as misc,
        ):
            bias_t = misc.tile([KR, 1], dt)
            nc.sync.dma_start(bias_t[:], bt[:])
            staging = misc.tile([KR, NG, N], dt)

            p0 = 0
            for ch, PC in enumerate(sizes):
                xtile = xfpool.tile([KR, PC, NQ, N + FOUT], mdt, tag="xf")
                sl = slice(p0, p0 + PC)
                eng = nc.sync if ch % 2 == 0 else nc.scalar
                eng.dma_start(xtile[:], xf[:, sl, :, :])
                for g in range(PC // 2):
                    gg = (p0 + g * 2) // 4       # psum group id (2 patches/iter)
                    half = (p0 + g * 2) % 4      # 0 or 2: which half of the group
                    if half == 0:
                        ptile = psum.tile([KR, N], dt, tag="ps")
                    for s2 in range(2):
                        s = half + s2
                        p = g * 2 + s2
                        for q in range(NQ):
                            nc.tensor.matmul(
                                ptile[32 * s : 32 * s + 32, :],
                                xtile[:, p, q, N : N + FOUT],  # lhsT [128,32(o)]
                                xtile[:, p, q, 0:N],           # rhs  [128,32(b)]
                                start=(q == 0),
                                stop=(q == NQ - 1),
                                tile_position=(0, 32 * s),
                            )
                    if half == 2:
                        nc.scalar.activation(
                            staging[:, gg, :], ptile[:], relu, bias=bias_t[:]
                        )
                        if (gg + 1) % GSPLIT == 0:
                            osl = slice(gg + 1 - GSPLIT, gg + 1)
                            oeng = nc.sync if gg + 1 == NG else nc.gpsimd
                            oeng.dma_start(out[:, osl, :], staging[:, osl, :])
                p0 += PC
    nc.compile()
    return nc


def _build_module_r(bufs=8):
    """float32r variant: single-pass fp32 matmuls (tf32-ish precision),
    PSUM packing along the free axis (8 patches per bank) since fp32r
    requires dst base partition 0. Half the PE instruction stream of the
    fp32 variant -> fewer IRAM paging stalls."""
    from concourse import bacc, tile, mybir

    nc = bacc.Bacc("TRN2", target_bir_lowering=False, debug=False, enable_asserts=False)
    dt = mybir.dt.float32
    mdt = mybir.dt.float32r
    SG = 8                      # patches per PSUM super-group
    NSG = PL // SG              # 16
    xf = nc.dram_tensor("xf", [KR, PL, NQ, N + FOUT], mdt, kind="ExternalInput").ap()
    bt = nc.dram_tensor("bt", [FOUT, 1], dt, kind="ExternalInput").ap()
    out = nc.dram_tensor("out", [FOUT, PL, N], dt, kind="ExternalOutput").ap()

    # Graduated [2,2,4] head (earliest first matmul; measured tightest
    # variance) and a [4,4] tail that halves the final
    # load->matmul->ACT->store chain.
    sizes = [2, 2, 4] + [8] * ((PL - 16) // 8) + [4, 2, 2]
    assert sum(sizes) == PL
    # PSUM eviction groups: 8-patch banks, except two 4-patch mini-groups
    # at the end so the last matmul->ACT->store chain is half as long.
    groups = [(g * SG, SG) for g in range(NSG - 1)] + [(PL - 8, 4), (PL - 4, 4)]
    gof = {}
    for gi, (s0, gsz) in enumerate(groups):
        for i in range(gsz):
            gof[s0 + i] = (gi, i)
    relu = mybir.ActivationFunctionType.Relu

    with tile.TileContext(nc) as tc:
        with (
            tc.tile_pool(name="xfpool", bufs=bufs) as xfpool,
            tc.tile_pool(name="psum", bufs=6, space="PSUM") as psum,
            tc.tile_pool(name="misc", bufs=1) as misc,
        ):
            # bias rides the scalar ring so it doesn't burn sync's first
            # DMA slot (~0.7 us of stream start).
            bias_t = misc.tile([FOUT, 1], dt)
            nc.scalar.dma_start(bias_t[:], bt[:])
            staging = misc.tile([FOUT, PL, N], dt)

            p0 = 0
            ptile = None
            for ch, PC in enumerate(sizes):
                xtile = xfpool.tile([KR, PC, NQ, N + FOUT], mdt, tag="xf")
                # All loads on sync's single HWDGE FIFO: strictly in-order
                # completions. (Arming chunk 0 on the scalar ring was tried
                # and is bimodal: when sync's big queue gets ahead, chunk 0
                # drains at round-robin half-rate and the in-order PE
                # consumption slips ~8 us.)
                nc.sync.dma_start(xtile[:], xf[:, p0 : p0 + PC, :, :])
                for pl in range(PC):
                    p = p0 + pl
                    gi, i = gof[p]
                    s0, gsz = groups[gi]
                    if i == 0:
                        ptile = psum.tile([FOUT, SG, N], dt, tag="ps")
                    for q in range(NQ):
                        nc.tensor.matmul(
                            ptile[:, i, :],
                            xtile[:, pl, q, N : N + FOUT],  # lhsT [128,32(o)]
                            xtile[:, pl, q, 0:N],           # rhs  [128,32(b)]
                            start=(q == 0),
                            stop=(q == NQ - 1),
                        )
                    if i == gsz - 1:
                        nc.scalar.activation(
                            staging[:, s0 : s0 + gsz, :],
                            ptile[:, :gsz, :],
                            relu,
                            bias=bias_t[:],
                        )
                        # Stores also ride the scalar ring, LAGGED two groups
                        # behind the ACT stream: their ACT dependency is long
                        # complete, so they never stall scalar (and the sync
                        # load ring is untouched). The final two stores are
                        # pure program-order after the last ACT.
                        if gi == len(groups) - 1:
                            a = groups[gi - 2][0]
                            nc.scalar.dma_start(
                                out[:, a:s0, :], staging[:, a:s0, :]
                            )
                            nc.scalar.dma_start(
                                out[:, s0:PL, :], staging[:, s0:PL, :]
                            )
                        elif gi % 2 == 1 and gi >= 3:
                            a = groups[gi - 3][0]
                            b = groups[gi - 1][0]
                            nc.scalar.dma_start(
                                out[:, a:b, :], staging[:, a:b, :]
                            )
                p0 += PC
    nc.compile()
    return nc


def _build_module_bf16(bufs=7, sizes=(32, 32, 24, 16, 8, 8, 8), act="scalar"):
    """bf16 variant: both operands cast to bf16 host-side (rel err ~3e-3,
    well under the 2e-2 gate) which halves HBM input traffic — the
    bottleneck (memory-bound problem). Output is stored bf16 too and
    upcast on the host. Bonus: bf16 matmuls run at 1 cycle/row on the PE
    vs fp32r's 4 at peak clock, so TensorE drops out of the picture.
    Same schedule skeleton as the fp32r variant."""
    from concourse import bacc, tile, mybir

    nc = bacc.Bacc("TRN2", target_bir_lowering=False, debug=False, enable_asserts=False)
    dt = mybir.dt.float32
    mdt = mybir.dt.bfloat16
    odt = mybir.dt.bfloat16
    SG = 8                      # patches per PSUM super-group
    NSG = PL // SG              # 16
    xf = nc.dram_tensor("xf", [KR, PL, NQ, N + FOUT], mdt, kind="ExternalInput").ap()
    bt = nc.dram_tensor("bt", [FOUT, 1], dt, kind="ExternalInput").ap()
    out = nc.dram_tensor("out", [FOUT, PL, N], odt, kind="ExternalOutput").ap()

    sizes = list(sizes)
    assert sum(sizes) == PL
    groups = [(g * SG, SG) for g in range(NSG - 1)] + [(PL - 8, 4), (PL - 4, 4)]
    gof = {}
    for gi, (s0, gsz) in enumerate(groups):
        for i in range(gsz):
            gof[s0 + i] = (gi, i)
    relu = mybir.ActivationFunctionType.Relu

    with tile.TileContext(nc) as tc:
        with (
            tc.tile_pool(name="xfpool", bufs=bufs) as xfpool,
            tc.tile_pool(name="psum", bufs=6, space="PSUM") as psum,
            tc.tile_pool(name="misc", bufs=1) as misc,
        ):
            # bias rides the scalar ring (a tiny DMA at the head of a ring
            # stalls that ring ~3us before its next bulk transfer — so it
            # must NOT share a ring with the input stream).
            bias_t = misc.tile([FOUT, 1], dt)
            nc.scalar.dma_start(bias_t[:], bt[:])
            staging = misc.tile([FOUT, PL, N], odt)

            # All input loads on sync's single HWDGE ring, issued upfront:
            # strictly in-order completions at ~390 GB/s. Few, large chunks
            # (each ring DMA costs ~1us of ring stall at its completion
            # boundary, and small chunks make the per-chunk engine stagger
            # dominate); one tiny final chunk keeps the last
            # data->matmul->ACT->store chain short.
            p0 = 0
            tiles = []
            for ch, PC in enumerate(sizes):
                xtile = xfpool.tile([KR, PC, NQ, N + FOUT], mdt, tag="xf")
                nc.sync.dma_start(xtile[:], xf[:, p0 : p0 + PC, :, :])
                tiles.append((xtile, p0, PC))
                p0 += PC

            ptile = None
            for ch, (xtile, p0, PC) in enumerate(tiles):
                for pl in range(PC):
                    p = p0 + pl
                    gi, i = gof[p]
                    s0, gsz = groups[gi]
                    if i == 0:
                        ptile = psum.tile([FOUT, SG, N], dt, tag="ps")
                    for q in range(NQ):
                        nc.tensor.matmul(
                            ptile[:, i, :],
                            xtile[:, pl, q, N : N + FOUT],  # lhsT [128,32(o)]
                            xtile[:, pl, q, 0:N],           # rhs  [128,32(b)]
                            start=(q == 0),
                            stop=(q == NQ - 1),
                        )
                    if i == gsz - 1:
                        if act == "scalar":
                            nc.scalar.activation(
                                staging[:, s0 : s0 + gsz, :],
                                ptile[:, :gsz, :],
                                relu,
                                bias=bias_t[:],
                            )
                        else:
                            # DVE: max(psum + bias, 0) in one instruction;
                            # no ACT table load, and the store issues on
                            # scalar no longer queue behind ACT execution.
                            nc.vector.tensor_scalar(
                                staging[:, s0 : s0 + gsz, :],
                                ptile[:, :gsz, :],
                                bias_t[:],
                                0.0,
                                mybir.AluOpType.add,
                                mybir.AluOpType.max,
                            )
                        # Stores ride the scalar ring. Bulk ones lag a few
                        # groups behind the ACT stream; near the end store
                        # eagerly so the final chain is just
                        # ACT -> one small store.
                        if gi == len(groups) - 1:
                            a = groups[gi - 1][0]
                            nc.scalar.dma_start(
                                out[:, a:PL, :], staging[:, a:PL, :]
                            )
                        elif gi == len(groups) - 3:
                            a = groups[7][0]
                            nc.scalar.dma_start(
                                out[:, a:s0 + gsz, :], staging[:, a:s0 + gsz, :]
                            )
                        elif gi == 7:
                            nc.scalar.dma_start(
                                out[:, 0:s0, :], staging[:, 0:s0, :]
                            )
    nc.compile()
    return nc


def _build_module_bf16_s(
    bulk_sizes=(32, 32, 32), tail_sizes=(16, 8, 8), bufs=3,
    fp8_first=False, last_store="scalar", packed=False, tail_bufs=None,
    warm=None, act_alt=False, store_late=False,
):
    """Hybrid-precision variant. Patches 0..sum(bulk_sizes) carry the X
    operand of their last two K-chunks (q=2,3) as fp8-e4m3 (filters stay
    bf16), cutting input traffic 12.5% on the bulk at a measured total
    rel err of ~1.6e-2 (< the 2e-2 gate; pure bf16 is 2.9e-3). The tail
    patches stay fully bf16 with the proven single-DMA-per-chunk layout
    so the end-of-stream chain is unchanged.

    Bulk chunks load three tensors each (bf16 q01 pack, bf16 q23
    filters, fp8 q23 X); all loads ride sync's ring in order. Semaphore-
    pool reuse waits land on sync's own issue stream only (benign: the
    ring stays several chunks deep)."""
    from concourse import bacc, tile, mybir

    nc = bacc.Bacc("TRN2", target_bir_lowering=False, debug=False, enable_asserts=False)
    dt = mybir.dt.float32
    mdt = mybir.dt.bfloat16
    f8 = mybir.dt.float8e4
    odt = mybir.dt.bfloat16
    SG = 8
    NSG = PL // SG
    BP = sum(bulk_sizes)              # bulk patch count
    TP = sum(tail_sizes)
    assert BP + TP == PL
    # xf1 packs, per (patch, qq in {0,1}): X_qq (32 cols) | F_qq (32) |
    # F_{qq+2} (32) — one bf16 stream; the q23 X rides separately as fp8.
    W1 = N + 2 * FOUT + (N // 2 if packed else 0)
    xf1 = nc.dram_tensor(
        "xf1", [KR, BP, 2, W1], mdt, kind="ExternalInput"
    ).ap()
    xf2x = (None if packed else
            nc.dram_tensor("xf2x", [KR, BP, 2, N], f8, kind="ExternalInput").ap())
    xft = nc.dram_tensor("xft", [KR, TP, NQ, N + FOUT], mdt, kind="ExternalInput").ap()
    bt = nc.dram_tensor("bt", [FOUT, 1], dt, kind="ExternalInput").ap()
    out = nc.dram_tensor("out", [FOUT, PL, N], odt, kind="ExternalOutput").ap()

    groups = [(g * SG, SG) for g in range(NSG - 1)] + [(PL - 8, 4), (PL - 4, 4)]
    gof = {}
    for gi, (s0, gsz) in enumerate(groups):
        for i in range(gsz):
            gof[s0 + i] = (gi, i)
    relu = mybir.ActivationFunctionType.Relu

    with tile.TileContext(nc) as tc:
        with (
            tc.tile_pool(name="xfpool", bufs=bufs) as xfpool,
            tc.tile_pool(name="psum", bufs=6, space="PSUM") as psum,
            tc.tile_pool(name="misc", bufs=1) as misc,
        ):
            bias_t = misc.tile([FOUT, 1], dt)
            nc.scalar.dma_start(bias_t[:], bt[:])
            staging = misc.tile([FOUT, PL, N], odt)

            # chunk list: (kind, tiles, p0, PC); bulk then tail.
            # (Moving the fp8 side-stream to the scalar ring as one DMA
            # was tried and measured ~1.3us WORSE: the dual-ring packet
            # interleave slows the bulk chunks' completions more than the
            # small per-chunk fp8 DMAs cost on sync's ring.)
            chunks = []
            t2x_all = None
            if fp8_first:
                # one fp8 DMA for all bulk patches at the head of sync's
                # ring: removes the per-chunk small-DMA handout stalls
                # mid-stream; the PE-start delay it adds is absorbed by
                # the PE's notify-paced idle.
                t2x_all = misc.tile([KR, BP, 2, N], f8)
                nc.sync.dma_start(t2x_all[:], xf2x[:])
            p0 = 0
            for PC in bulk_sizes:
                t1 = xfpool.tile([KR, PC, 2, W1], mdt, tag="x1")
                nc.sync.dma_start(t1[:], xf1[:, p0 : p0 + PC, :, :])
                if packed:
                    chunks.append(("b", (t1, None, None), p0, PC))
                elif fp8_first:
                    chunks.append(("b", (t1, t2x_all, p0), p0, PC))
                else:
                    t2x = xfpool.tile([KR, PC, 2, N], f8, tag="x2x")
                    nc.sync.dma_start(t2x[:], xf2x[:, p0 : p0 + PC, :, :])
                    chunks.append(("b", (t1, t2x, None), p0, PC))
                p0 += PC
            q0 = 0
            for PC in tail_sizes:
                tt = xfpool.tile(
                    [KR, PC, NQ, N + FOUT], mdt, tag="xt",
                    bufs=tail_bufs if tail_bufs else None,
                )
                nc.sync.dma_start(tt[:], xft[:, q0 : q0 + PC, :, :])
                chunks.append(("t", (tt,), BP + q0, PC))
                q0 += PC

            warm = dict(warm or {})
            wscr = None
            if warm:
                wscr = psum.tile([FOUT, 384], dt, tag="warm", bufs=1)
            ptile = None
            for kind, tiles, p0, PC in chunks:
                for pl in range(PC):
                    p = p0 + pl
                    if p in warm:
                        # HAM warm-keeper: wide dummy matmuls with no
                        # unmet deps fill the I$-refill stall just ahead,
                        # so the PE activity window never goes idle long
                        # enough to drop the clock to 1.2 GHz — the tail
                        # then runs at the full 2.4 GHz.
                        wt = tiles[0]
                        wl = wt[:, 0, 0, N : N + FOUT]
                        wr = wt[:, 0:2, :, :]
                        for _ in range(warm[p]):
                            nc.tensor.matmul(
                                wscr[:, : wr.free_size()], wl, wr,
                                start=True, stop=True,
                                skip_group_check=True,
                            )
                    gi, i = gof[p]
                    s0, gsz = groups[gi]
                    if i == 0:
                        ptile = psum.tile([FOUT, SG, N], dt, tag="ps")
                    for q in range(NQ):
                        if kind == "t":
                            tt = tiles[0]
                            lhsT = tt[:, pl, q, N : N + FOUT]
                            rhs = tt[:, pl, q, 0:N]
                        elif q < 2:
                            t1 = tiles[0]
                            lhsT = t1[:, pl, q, N : N + FOUT]
                            rhs = t1[:, pl, q, 0:N]
                        elif tiles[1] is None:
                            t1 = tiles[0]
                            lhsT = t1[:, pl, q - 2, N + FOUT : N + 2 * FOUT]
                            rhs = t1[
                                :, pl, q - 2, N + 2 * FOUT : W1
                            ].bitcast(f8)
                        else:
                            lhsT = tiles[0][:, pl, q - 2, N + FOUT : N + 2 * FOUT]
                            ri = pl if tiles[2] is None else p
                            rhs = tiles[1][:, ri, q - 2, :]
                        nc.tensor.matmul(
                            ptile[:, i, :],
                            lhsT,
                            rhs,
                            start=(q == 0),
                            stop=(q == NQ - 1),
                        )
                    if i == gsz - 1:
                        if act_alt and gi % 2 == 1:
                            # alternate evictions onto the DVE so the
                            # final few PSUM groups drain on two engines
                            # in parallel instead of serializing on
                            # ScalarE behind the ACT stream
                            nc.vector.tensor_scalar(
                                staging[:, s0 : s0 + gsz, :],
                                ptile[:, :gsz, :],
                                bias_t[:],
                                0.0,
                                mybir.AluOpType.add,
                                mybir.AluOpType.max,
                            )
                        else:
                            nc.scalar.activation(
                                staging[:, s0 : s0 + gsz, :],
                                ptile[:, :gsz, :],
                                relu,
                                bias=bias_t[:],
                            )
                        if store_late:
                            # Defer bulk stores to the stream's end: their
                            # engine-time stops interleaving with input
                            # packets (which delays the last chunks'
                            # completion sems); they drain instead in the
                            # shadow of the PE/ACT tail.
                            if gi == len(groups) - 1:
                                nc.scalar.dma_start(
                                    out[:, 124:PL, :], staging[:, 124:PL, :]
                                )
                            elif gi == len(groups) - 2:
                                nc.scalar.dma_start(
                                    out[:, 56:124, :], staging[:, 56:124, :]
                                )
                            elif gi == 13:
                                nc.scalar.dma_start(
                                    out[:, 0:56, :], staging[:, 0:56, :]
                                )
                        elif gi == len(groups) - 1:
                            a = groups[gi - 1][0]
                            eng = nc.gpsimd if last_store == "gpsimd" else nc.scalar
                            eng.dma_start(
                                out[:, a:PL, :], staging[:, a:PL, :]
                            )
                        elif gi == len(groups) - 3:
                            a = groups[7][0]
                            nc.scalar.dma_start(
                                out[:, a : s0 + gsz, :],
                                staging[:, a : s0 + gsz, :],
                            )
                        elif gi == 7:
                            nc.scalar.dma_start(
                                out[:, 0:s0, :], staging[:, 0:s0, :]
                            )
    nc.compile()
    return nc


def _build_module_bf16_s2(
    bulk_sizes=(32, 32, 32), tail_sizes=(16, 8, 6, 2), bufs=3,
    last_on_scalar=False,
):
    """Packed hybrid-precision variant. Like bf16s, but each bulk chunk is
    ONE DMA: the q23 X fp8 bytes ride embedded at the end of each bulk row
    (W1 = 32 X-bf16 + 64 F-bf16 + 16 slots holding 32 fp8 X), so the sync
    ring carries 3 bulk + len(tail_sizes) tail DMAs total. The tail is
    graduated down to a 2-patch final chunk so the last
    data->matmul->ACT->store chain is minimal. Optionally the final tiny
    chunk rides the scalar ring, issued upfront, so its data and notify
    land mid-stream and the PE's last gate is the penultimate chunk."""
    from concourse import bacc, tile, mybir

    nc = bacc.Bacc("TRN2", target_bir_lowering=False, debug=False, enable_asserts=False)
    dt = mybir.dt.float32
    mdt = mybir.dt.bfloat16
    f8 = mybir.dt.float8e4
    odt = mybir.dt.bfloat16
    SG = 8
    NSG = PL // SG
    BP = sum(bulk_sizes)
    TP = sum(tail_sizes)
    assert BP + TP == PL
    W1 = N + 2 * FOUT + N // 2        # 112 bf16 cols per (patch, qq) row
    xf1 = nc.dram_tensor("xf1", [KR, BP, 2, W1], mdt, kind="ExternalInput").ap()
    xft = nc.dram_tensor("xft", [KR, TP, NQ, N + FOUT], mdt, kind="ExternalInput").ap()
    bt = nc.dram_tensor("bt", [FOUT, 1], dt, kind="ExternalInput").ap()
    out = nc.dram_tensor("out", [FOUT, PL, N], odt, kind="ExternalOutput").ap()

    groups = [(g * SG, SG) for g in range(NSG - 1)] + [(PL - 8, 4), (PL - 4, 4)]
    gof = {}
    for gi, (s0, gsz) in enumerate(groups):
        for i in range(gsz):
            gof[s0 + i] = (gi, i)
    relu = mybir.ActivationFunctionType.Relu
    ntail = len(tail_sizes)

    with tile.TileContext(nc) as tc:
        with (
            tc.tile_pool(name="xfpool", bufs=bufs) as xfpool,
            tc.tile_pool(name="psum", bufs=6, space="PSUM") as psum,
            tc.tile_pool(name="misc", bufs=1) as misc,
        ):
            bias_t = misc.tile([FOUT, 1], dt)
            staging = misc.tile([FOUT, PL, N], odt)

            chunks = []
            p0 = 0
            for PC in bulk_sizes:
                t1 = xfpool.tile([KR, PC, 2, W1], mdt, tag="x1")
                nc.sync.dma_start(t1[:], xf1[:, p0 : p0 + PC, :, :])
                chunks.append(("b", t1, p0, PC))
                p0 += PC
            q0 = 0
            tail_chunks = []
            for ti, PC in enumerate(tail_sizes):
                tt = xfpool.tile(
                    [KR, PC, NQ, N + FOUT], mdt, tag="xt", bufs=ntail
                )
                if last_on_scalar and ti == ntail - 1:
                    nc.scalar.dma_start(tt[:], xft[:, q0 : q0 + PC, :, :])
                else:
                    nc.sync.dma_start(tt[:], xft[:, q0 : q0 + PC, :, :])
                tail_chunks.append(("t", tt, BP + q0, PC))
                q0 += PC
            chunks += tail_chunks
            # bias AFTER the loads on the scalar ring (a tiny DMA at a
            # ring head stalls that ring ~3us before its next bulk move).
            nc.scalar.dma_start(bias_t[:], bt[:])

            ptile = None
            for kind, tt, p0, PC in chunks:
                for pl in range(PC):
                    p = p0 + pl
                    gi, i = gof[p]
                    s0, gsz = groups[gi]
                    if i == 0:
                        ptile = psum.tile([FOUT, SG, N], dt, tag="ps")
                    for q in range(NQ):
                        if kind == "t":
                            lhsT = tt[:, pl, q, N : N + FOUT]
                            rhs = tt[:, pl, q, 0:N]
                        elif q < 2:
                            lhsT = tt[:, pl, q, N : N + FOUT]
                            rhs = tt[:, pl, q, 0:N]
                        else:
                            lhsT = tt[:, pl, q - 2, N + FOUT : N + 2 * FOUT]
                            rhs = tt[:, pl, q - 2, N + 2 * FOUT : W1].bitcast(f8)
                        nc.tensor.matmul(
                            ptile[:, i, :],
                            lhsT,
                            rhs,
                            start=(q == 0),
                            stop=(q == NQ - 1),
                        )
                    if i == gsz - 1:
                        nc.scalar.activation(
                            staging[:, s0 : s0 + gsz, :],
                            ptile[:, :gsz, :],
                            relu,
                            bias=bias_t[:],
                        )
                        if gi == len(groups) - 1:
                            a = groups[gi - 1][0]
                            nc.scalar.dma_start(
                                out[:, a:PL, :], staging[:, a:PL, :]
                            )
                        elif gi == len(groups) - 3:
                            a = groups[7][0]
                            nc.scalar.dma_start(
                                out[:, a : s0 + gsz, :],
                                staging[:, a : s0 + gsz, :],
                            )
                        elif gi == 7:
                            nc.scalar.dma_start(
                                out[:, 0:s0, :], staging[:, 0:s0, :]
                            )
    nc.compile()
    return nc


def _build_module_bf16_s4(sizes=(32, 32, 32, 16, 8, 8), bufs=None):
    """Full-coverage hybrid precision: q23 X rides fp8 for ALL 128 patches
    (measured rel err 1.888e-2 vs the 2e-2 gate; 96-patch coverage was
    1.64e-2). Uniform chunk layout: per chunk a bf16 DMA (X_q01 | F_q01 |
    F_q23) and an fp8 DMA (X_q23), all on sync's ring in order. Saves
    0.26 MB of input traffic vs bf16s."""
    from concourse import bacc, tile, mybir

    nc = bacc.Bacc("TRN2", target_bir_lowering=False, debug=False, enable_asserts=False)
    dt = mybir.dt.float32
    mdt = mybir.dt.bfloat16
    f8 = mybir.dt.float8e4
    odt = mybir.dt.bfloat16
    SG = 8
    NSG = PL // SG
    W1 = N + 2 * FOUT                 # 96 bf16 cols per (patch, qq) row
    assert sum(sizes) == PL
    nch = len(sizes)
    if bufs is None:
        bufs = nch
    xf1 = nc.dram_tensor("xf1", [KR, PL, 2, W1], mdt, kind="ExternalInput").ap()
    xf2x = nc.dram_tensor("xf2x", [KR, PL, 2, N], f8, kind="ExternalInput").ap()
    bt = nc.dram_tensor("bt", [FOUT, 1], dt, kind="ExternalInput").ap()
    out = nc.dram_tensor("out", [FOUT, PL, N], odt, kind="ExternalOutput").ap()

    groups = [(g * SG, SG) for g in range(NSG - 1)] + [(PL - 8, 4), (PL - 4, 4)]
    gof = {}
    for gi, (s0, gsz) in enumerate(groups):
        for i in range(gsz):
            gof[s0 + i] = (gi, i)
    relu = mybir.ActivationFunctionType.Relu

    with tile.TileContext(nc) as tc:
        with (
            tc.tile_pool(name="xfpool", bufs=bufs) as xfpool,
            tc.tile_pool(name="psum", bufs=6, space="PSUM") as psum,
            tc.tile_pool(name="misc", bufs=1) as misc,
        ):
            bias_t = misc.tile([FOUT, 1], dt)
            nc.scalar.dma_start(bias_t[:], bt[:])
            staging = misc.tile([FOUT, PL, N], odt)

            chunks = []
            p0 = 0
            for PC in sizes:
                t1 = xfpool.tile([KR, PC, 2, W1], mdt, tag="x1")
                nc.sync.dma_start(t1[:], xf1[:, p0 : p0 + PC, :, :])
                t2x = xfpool.tile([KR, PC, 2, N], f8, tag="x2x")
                nc.sync.dma_start(t2x[:], xf2x[:, p0 : p0 + PC, :, :])
                chunks.append((t1, t2x, p0, PC))
                p0 += PC

            ptile = None
            for t1, t2x, p0, PC in chunks:
                for pl in range(PC):
                    p = p0 + pl
                    gi, i = gof[p]
                    s0, gsz = groups[gi]
                    if i == 0:
                        ptile = psum.tile([FOUT, SG, N], dt, tag="ps")
                    for q in range(NQ):
                        if q < 2:
                            lhsT = t1[:, pl, q, N : N + FOUT]
                            rhs = t1[:, pl, q, 0:N]
                        else:
                            lhsT = t1[:, pl, q - 2, N + FOUT : N + 2 * FOUT]
                            rhs = t2x[:, pl, q - 2, :]
                        nc.tensor.matmul(
                            ptile[:, i, :],
                            lhsT,
                            rhs,
                            start=(q == 0),
                            stop=(q == NQ - 1),
                        )
                    if i == gsz - 1:
                        nc.scalar.activation(
                            staging[:, s0 : s0 + gsz, :],
                            ptile[:, :gsz, :],
                            relu,
                            bias=bias_t[:],
                        )
                        if gi == len(groups) - 1:
                            a = groups[gi - 1][0]
                            nc.scalar.dma_start(
                                out[:, a:PL, :], staging[:, a:PL, :]
                            )
                        elif gi == len(groups) - 3:
                            a = groups[7][0]
                            nc.scalar.dma_start(
                                out[:, a : s0 + gsz, :],
                                staging[:, a : s0 + gsz, :],
                            )
                        elif gi == 7:
                            nc.scalar.dma_start(
                                out[:, 0:s0, :], staging[:, 0:s0, :]
                            )
    nc.compile()
    return nc


def _build_module_bf16_s5(sizes=(32, 32, 32, 16, 8, 8)):
    """Full-coverage fp8 with a single upfront fp8 mega-DMA. Uniform bf16
    chunk layout (X_q01 | F_q01 | F_q23) for all 128 patches; the q23 X
    fp8 bytes for the WHOLE core ride one 1.05 MB DMA at the head of
    sync's ring. 7 input DMAs total; 7.34 MB/core."""
    from concourse import bacc, tile, mybir

    nc = bacc.Bacc("TRN2", target_bir_lowering=False, debug=False, enable_asserts=False)
    dt = mybir.dt.float32
    mdt = mybir.dt.bfloat16
    f8 = mybir.dt.float8e4
    odt = mybir.dt.bfloat16
    SG = 8
    NSG = PL // SG
    W1 = N + 2 * FOUT
    assert sum(sizes) == PL
    xf1 = nc.dram_tensor("xf1", [KR, PL, 2, W1], mdt, kind="ExternalInput").ap()
    xf2x = nc.dram_tensor("xf2x", [KR, PL, 2, N], f8, kind="ExternalInput").ap()
    bt = nc.dram_tensor("bt", [FOUT, 1], dt, kind="ExternalInput").ap()
    out = nc.dram_tensor("out", [FOUT, PL, N], odt, kind="ExternalOutput").ap()

    groups = [(g * SG, SG) for g in range(NSG - 1)] + [(PL - 8, 4), (PL - 4, 4)]
    gof = {}
    for gi, (s0, gsz) in enumerate(groups):
        for i in range(gsz):
            gof[s0 + i] = (gi, i)
    relu = mybir.ActivationFunctionType.Relu

    with tile.TileContext(nc) as tc:
        with (
            tc.tile_pool(name="xfpool", bufs=len(sizes)) as xfpool,
            tc.tile_pool(name="psum", bufs=6, space="PSUM") as psum,
            tc.tile_pool(name="misc", bufs=1) as misc,
        ):
            bias_t = misc.tile([FOUT, 1], dt)
            nc.scalar.dma_start(bias_t[:], bt[:])
            staging = misc.tile([FOUT, PL, N], odt)

            t2x_all = misc.tile([KR, PL, 2, N], f8)
            nc.sync.dma_start(t2x_all[:], xf2x[:])
            chunks = []
            p0 = 0
            for PC in sizes:
                t1 = xfpool.tile([KR, PC, 2, W1], mdt, tag="x1")
                nc.sync.dma_start(t1[:], xf1[:, p0 : p0 + PC, :, :])
                chunks.append((t1, p0, PC))
                p0 += PC

            ptile = None
            for t1, p0, PC in chunks:
                for pl in range(PC):
                    p = p0 + pl
                    gi, i = gof[p]
                    s0, gsz = groups[gi]
                    if i == 0:
                        ptile = psum.tile([FOUT, SG, N], dt, tag="ps")
                    for q in range(NQ):
                        if q < 2:
                            lhsT = t1[:, pl, q, N : N + FOUT]
                            rhs = t1[:, pl, q, 0:N]
                        else:
                            lhsT = t1[:, pl, q - 2, N + FOUT : N + 2 * FOUT]
                            rhs = t2x_all[:, p, q - 2, :]
                        nc.tensor.matmul(
                            ptile[:, i, :],
                            lhsT,
                            rhs,
                            start=(q == 0),
                            stop=(q == NQ - 1),
                        )
                    if i == gsz - 1:
                        nc.scalar.activation(
                            staging[:, s0 : s0 + gsz, :],
                            ptile[:, :gsz, :],
                            relu,
                            bias=bias_t[:],
                        )
                        if gi == len(groups) - 1:
                            a = groups[gi - 1][0]
                            nc.scalar.dma_start(
                                out[:, a:PL, :], staging[:, a:PL, :]
                            )
                        elif gi == len(groups) - 3:
                            a = groups[7][0]
                            nc.scalar.dma_start(
                                out[:, a : s0 + gsz, :],
                                staging[:, a : s0 + gsz, :],
                            )
                        elif gi == 7:
                            nc.scalar.dma_start(
                                out[:, 0:s0, :], staging[:, 0:s0, :]
                            )
    nc.compile()
    return nc


def _build_module_bf16_s7(
    bulk_sizes=(32, 32, 32), tail_sizes=(16, 8, 8), bufs=3,
):
    """Full-coverage fp8 keeping the proven bf16s stream shape. Bulk
    chunks: per-chunk bf16 (X_q01|F_q01|F_q23) + fp8 (X_q23) DMAs, as in
    bf16s. Tail chunks: bf16 part per-chunk, but their q23-X fp8 rides ONE
    early 256 KB DMA (rows 2 KB) right after chunk 0's pair, so no
    sub-line-rate descriptors appear at the stream end. 10 input DMAs,
    7.34 MB/core, rel err 1.888e-2."""
    from concourse import bacc, tile, mybir

    nc = bacc.Bacc("TRN2", target_bir_lowering=False, debug=False, enable_asserts=False)
    dt = mybir.dt.float32
    mdt = mybir.dt.bfloat16
    f8 = mybir.dt.float8e4
    odt = mybir.dt.bfloat16
    SG = 8
    NSG = PL // SG
    BP = sum(bulk_sizes)
    TP = sum(tail_sizes)
    assert BP + TP == PL
    W1 = N + 2 * FOUT
    xf1 = nc.dram_tensor("xf1", [KR, PL, 2, W1], mdt, kind="ExternalInput").ap()
    xf2x = nc.dram_tensor("xf2x", [KR, PL, 2, N], f8, kind="ExternalInput").ap()
    bt = nc.dram_tensor("bt", [FOUT, 1], dt, kind="ExternalInput").ap()
    out = nc.dram_tensor("out", [FOUT, PL, N], odt, kind="ExternalOutput").ap()

    groups = [(g * SG, SG) for g in range(NSG - 1)] + [(PL - 8, 4), (PL - 4, 4)]
    gof = {}
    for gi, (s0, gsz) in enumerate(groups):
        for i in range(gsz):
            gof[s0 + i] = (gi, i)
    relu = mybir.ActivationFunctionType.Relu
    ntail = len(tail_sizes)

    with tile.TileContext(nc) as tc:
        with (
            tc.tile_pool(name="xfpool", bufs=bufs) as xfpool,
            tc.tile_pool(name="psum", bufs=6, space="PSUM") as psum,
            tc.tile_pool(name="misc", bufs=1) as misc,
        ):
            bias_t = misc.tile([FOUT, 1], dt)
            nc.scalar.dma_start(bias_t[:], bt[:])
            staging = misc.tile([FOUT, PL, N], odt)
            t2x_tail = misc.tile([KR, TP, 2, N], f8)

            chunks = []
            p0 = 0
            for ci, PC in enumerate(bulk_sizes):
                t1 = xfpool.tile([KR, PC, 2, W1], mdt, tag="x1")
                nc.sync.dma_start(t1[:], xf1[:, p0 : p0 + PC, :, :])
                t2x = xfpool.tile([KR, PC, 2, N], f8, tag="x2x")
                nc.sync.dma_start(t2x[:], xf2x[:, p0 : p0 + PC, :, :])
                chunks.append((t1, t2x, None, p0, PC))
                if ci == 0:
                    # tail fp8 early: small, lands mid-stream, and keeps
                    # the stream's end free of sub-KB descriptors
                    nc.sync.dma_start(t2x_tail[:], xf2x[:, BP:PL, :, :])
                p0 += PC
            q0 = 0
            for PC in tail_sizes:
                tt = xfpool.tile(
                    [KR, PC, 2, W1], mdt, tag="xt", bufs=ntail
                )
                nc.sync.dma_start(tt[:], xf1[:, BP + q0 : BP + q0 + PC, :, :])
                chunks.append((tt, t2x_tail, q0, BP + q0, PC))
                q0 += PC

            ptile = None
            for t1, t2x, toff, p0, PC in chunks:
                for pl in range(PC):
                    p = p0 + pl
                    gi, i = gof[p]
                    s0, gsz = groups[gi]
                    if i == 0:
                        ptile = psum.tile([FOUT, SG, N], dt, tag="ps")
                    for q in range(NQ):
                        if q < 2:
                            lhsT = t1[:, pl, q, N : N + FOUT]
                            rhs = t1[:, pl, q, 0:N]
                        else:
                            lhsT = t1[:, pl, q - 2, N + FOUT : N + 2 * FOUT]
                            ri = pl if toff is None else toff + pl
                            rhs = t2x[:, ri, q - 2, :]
                        nc.tensor.matmul(
                            ptile[:, i, :],
                            lhsT,
                            rhs,
                            start=(q == 0),
                            stop=(q == NQ - 1),
                        )
                    if i == gsz - 1:
                        nc.scalar.activation(
                            staging[:, s0 : s0 + gsz, :],
                            ptile[:, :gsz, :],
                            relu,
                            bias=bias_t[:],
                        )
                        if gi == len(groups) - 1:
                            a = groups[gi - 1][0]
                            nc.scalar.dma_start(
                                out[:, a:PL, :], staging[:, a:PL, :]
                            )
                        elif gi == len(groups) - 3:
                            a = groups[7][0]
                            nc.scalar.dma_start(
                                out[:, a : s0 + gsz, :],
                                staging[:, a : s0 + gsz, :],
                            )
                        elif gi == 7:
                            nc.scalar.dma_start(
                                out[:, 0:s0, :], staging[:, 0:s0, :]
                            )
    nc.compile()
    return nc


def _marshal_s4(X, filters, bias):
    X = np.ascontiguousarray(np.asarray(X, dtype=np.float32))
    filters = np.ascontiguousarray(np.asarray(filters, dtype=np.float32))
    bias = np.asarray(bias, dtype=np.float32)
    xv = X.reshape(N, NCORES, 4, FH, 32, FW, C)
    xt = xv.transpose(1, 5, 6, 2, 4, 3, 0).reshape(NCORES, KR, PL, NQ, N)
    fv = filters.reshape(NCORES, PL, FH, FW, C, FOUT)
    ft = fv.transpose(0, 3, 4, 1, 2, 5).reshape(NCORES, KR, PL, NQ, FOUT)
    bf16 = ml_dtypes.bfloat16
    f8 = ml_dtypes.float8_e4m3fn
    xf1 = np.ascontiguousarray(
        np.concatenate(
            [xt[:, :, :, :2, :], ft[:, :, :, :2, :], ft[:, :, :, 2:, :]],
            axis=4,
        ).astype(bf16)
    )
    xf2x = np.ascontiguousarray(xt[:, :, :, 2:, :].astype(f8))
    bt = np.ascontiguousarray(bias.reshape(FOUT, 1))
    return xf1, xf2x, bt


def _build_module_bf16_t(a_sizes=(32, 32, 32, 16), b_size=16, bufs=5):
    """bf16 with the tail patches carried by the scalar ring, issued
    upfront: their data lands mid-stream, so the PE's final wait is only
    for the LAST sync-ring chunk; the B patches fill the completion-lag
    bubble before it. Processing order: A chunks ..., B, last A chunk."""
    from concourse import bacc, tile, mybir

    nc = bacc.Bacc("TRN2", target_bir_lowering=False, debug=False, enable_asserts=False)
    dt = mybir.dt.float32
    mdt = mybir.dt.bfloat16
    odt = mybir.dt.bfloat16
    SG = 8
    NSG = PL // SG
    xf = nc.dram_tensor("xf", [KR, PL, NQ, N + FOUT], mdt, kind="ExternalInput").ap()
    bt = nc.dram_tensor("bt", [FOUT, 1], dt, kind="ExternalInput").ap()
    out = nc.dram_tensor("out", [FOUT, PL, N], odt, kind="ExternalOutput").ap()

    a_sizes = list(a_sizes)
    assert sum(a_sizes) + b_size == PL
    groups = [(g * SG, SG) for g in range(NSG - 1)] + [(PL - 8, 4), (PL - 4, 4)]
    gof = {}
    for gi, (s0, gsz) in enumerate(groups):
        for i in range(gsz):
            gof[s0 + i] = (gi, i)
    relu = mybir.ActivationFunctionType.Relu

    a_edge = PL - b_size          # start of B's patch range
    # chunks in EMISSION order for loads; processing order reorders below
    with tile.TileContext(nc) as tc:
        with (
            tc.tile_pool(name="xfpool", bufs=bufs) as xfpool,
            tc.tile_pool(name="psum", bufs=6, space="PSUM") as psum,
            tc.tile_pool(name="misc", bufs=1) as misc,
        ):
            staging = misc.tile([FOUT, PL, N], odt)
            bias_t = misc.tile([FOUT, 1], dt)

            # loads: A chunks on sync upfront; B chunk on scalar FIRST
            # (a tiny DMA at a ring head stalls the ring ~3us, so bias
            # rides scalar AFTER the bulk B chunk).
            chunks = []    # (xtile, p0, PC) keyed by patch range
            p0 = 0
            for PC in a_sizes:
                xtile = xfpool.tile([KR, PC, NQ, N + FOUT], mdt, tag="xf")
                nc.sync.dma_start(xtile[:], xf[:, p0 : p0 + PC, :, :])
                chunks.append((xtile, p0, PC))
                p0 += PC
            btile = xfpool.tile([KR, b_size, NQ, N + FOUT], mdt, tag="xf")
            nc.scalar.dma_start(btile[:], xf[:, a_edge:PL, :, :])
            nc.scalar.dma_start(bias_t[:], bt[:])

            # processing order: all A chunks but the last, then B, then
            # the last A chunk
            order = chunks[:-1] + [(btile, a_edge, b_size)] + [chunks[-1]]
            last_gi = gof[chunks[-1][1] + chunks[-1][2] - 1][0]

            ptile = None
            for xtile, p0, PC in order:
                for pl in range(PC):
                    p = p0 + pl
                    gi, i = gof[p]
                    s0, gsz = groups[gi]
                    if i == 0:
                        ptile = psum.tile([FOUT, SG, N], dt, tag="ps")
                    for q in range(NQ):
                        nc.tensor.matmul(
                            ptile[:, i, :],
                            xtile[:, pl, q, N : N + FOUT],
                            xtile[:, pl, q, 0:N],
                            start=(q == 0),
                            stop=(q == NQ - 1),
                        )
                    if i == gsz - 1:
                        nc.scalar.activation(
                            staging[:, s0 : s0 + gsz, :],
                            ptile[:, :gsz, :],
                            relu,
                            bias=bias_t[:],
                        )
                        if gi == 11:
                            nc.scalar.dma_start(
                                out[:, 0:96, :], staging[:, 0:96, :]
                            )
                        elif gi == len(groups) - 1:
                            nc.scalar.dma_start(
                                out[:, a_edge:PL, :], staging[:, a_edge:PL, :]
                            )
                        elif gi == last_gi:
                            nc.scalar.dma_start(
                                out[:, 96:a_edge, :], staging[:, 96:a_edge, :]
                            )
    nc.compile()
    return nc


def _get_module():
    if "nc" not in _CACHE:
        _CACHE["nc"] = _build_module()
    return _CACHE["nc"]


def _marshal(X, filters, bias, mdtype=np.float32):
    """Shard + lay out full inputs into per-core device arrays."""
    X = np.ascontiguousarray(np.asarray(X, dtype=np.float32))
    filters = np.ascontiguousarray(np.asarray(filters, dtype=np.float32))
    bias = np.asarray(bias, dtype=np.float32)

    # X: (b, core, pr, i, pc, j, c) -> (core, j, c, pr, pc, i, b)
    xv = X.reshape(N, NCORES, 4, FH, 32, FW, C)
    xt = xv.transpose(1, 5, 6, 2, 4, 3, 0).reshape(NCORES, KR, PL, NQ, N)
    # filters: (core, p, i, j, c, o) -> (core, j, c, p, i, o)
    fv = filters.reshape(NCORES, PL, FH, FW, C, FOUT)
    ft = fv.transpose(0, 3, 4, 1, 2, 5).reshape(NCORES, KR, PL, NQ, FOUT)
    xfa = np.concatenate([xt, ft], axis=4)
    if mdtype != np.float32:
        xfa = xfa.astype(mdtype)
    xfa = np.ascontiguousarray(xfa)
    bt = np.ascontiguousarray(np.tile(bias, 4).reshape(KR, 1))
    return xfa, bt


def _assemble(outs):
    """Per-core out [128=(s,o), NG, N] -> full (N, 32, 32, FOUT)."""
    z = np.stack(outs)                                  # (core, (s,o), g, b)
    z = z.reshape(NCORES, 4, FOUT, NG, N)               # (core, s, o, g, b)
    z = z.transpose(4, 0, 3, 1, 2)                      # (b, core, g, s, o)
    z = z.reshape(N, NCORES, PL, FOUT)                  # p_loc = 4*g + s
    z = z.reshape(N, NCORES * 4, 32, FOUT)              # (b, pr_glob, pc, o)
    return np.ascontiguousarray(z)


S_BULK = 96          # patches with q23-X in fp8 (see _build_module_bf16_s)


def _marshal_s(X, filters, bias):
    X = np.ascontiguousarray(np.asarray(X, dtype=np.float32))
    filters = np.ascontiguousarray(np.asarray(filters, dtype=np.float32))
    bias = np.asarray(bias, dtype=np.float32)
    xv = X.reshape(N, NCORES, 4, FH, 32, FW, C)
    xt = xv.transpose(1, 5, 6, 2, 4, 3, 0).reshape(NCORES, KR, PL, NQ, N)
    fv = filters.reshape(NCORES, PL, FH, FW, C, FOUT)
    ft = fv.transpose(0, 3, 4, 1, 2, 5).reshape(NCORES, KR, PL, NQ, FOUT)
    BP = S_BULK
    bf16 = ml_dtypes.bfloat16
    f8 = ml_dtypes.float8_e4m3fn
    xf1 = np.ascontiguousarray(
        np.concatenate(
            [xt[:, :, :BP, :2, :], ft[:, :, :BP, :2, :], ft[:, :, :BP, 2:, :]],
            axis=4,
        ).astype(bf16)
    )
    xf2x = np.ascontiguousarray(xt[:, :, :BP, 2:, :].astype(f8))
    xft = np.ascontiguousarray(
        np.concatenate([xt[:, :, BP:, :, :], ft[:, :, BP:, :, :]], axis=4)
        .astype(bf16)
    )
    bt = np.ascontiguousarray(bias.reshape(FOUT, 1))
    return xf1, xf2x, xft, bt


def _marshal_s2(X, filters, bias):
    """Packed marshal: bulk rows [X_qq bf16 | F_qq | F_{qq+2} | X_{qq+2} fp8]."""
    X = np.ascontiguousarray(np.asarray(X, dtype=np.float32))
    filters = np.ascontiguousarray(np.asarray(filters, dtype=np.float32))
    bias = np.asarray(bias, dtype=np.float32)
    xv = X.reshape(N, NCORES, 4, FH, 32, FW, C)
    xt = xv.transpose(1, 5, 6, 2, 4, 3, 0).reshape(NCORES, KR, PL, NQ, N)
    fv = filters.reshape(NCORES, PL, FH, FW, C, FOUT)
    ft = fv.transpose(0, 3, 4, 1, 2, 5).reshape(NCORES, KR, PL, NQ, FOUT)
    BP = S_BULK
    W1 = N + 2 * FOUT + N // 2
    bf16 = ml_dtypes.bfloat16
    f8 = ml_dtypes.float8_e4m3fn
    a = np.ascontiguousarray(
        np.concatenate(
            [xt[:, :, :BP, :2, :], ft[:, :, :BP, :2, :], ft[:, :, :BP, 2:, :]],
            axis=4,
        ).astype(bf16)
    )
    x8 = np.ascontiguousarray(xt[:, :, :BP, 2:, :].astype(f8))
    xf1 = np.empty((NCORES, KR, BP, 2, W1), dtype=np.uint16)
    xf1[..., : N + 2 * FOUT] = a.view(np.uint16)
    xf1[..., N + 2 * FOUT :] = x8.view(np.uint16)
    xf1 = xf1.view(bf16)
    xft = np.ascontiguousarray(
        np.concatenate([xt[:, :, BP:, :, :], ft[:, :, BP:, :, :]], axis=4)
        .astype(bf16)
    )
    bt = np.ascontiguousarray(bias.reshape(FOUT, 1))
    return xf1, xft, bt


def _assemble_r(outs):
    """Per-core out [FOUT, PL, N] -> full (N, 32, 32, FOUT)."""
    z = np.stack(outs)                                  # (core, o, p, b)
    z = z.transpose(3, 0, 2, 1)                         # (b, core, p, o)
    return np.ascontiguousarray(z.reshape(N, 32, 32, FOUT))


LAST_RESULT = None
VARIANT = "bf16sc"


def kernel(X, filters, bias):
    global LAST_RESULT
    from concourse import bass_utils
    from concourse.bass_utils import run_bass_kernel_spmd

    # If tracing is enabled in the environment, keep the artifact upload
    # local so a missing bucket can't fail the run.
    bass_utils.upload_artifacts = lambda tmpdir: f"local://{tmpdir}"

    if "nc" not in _CACHE:
        _CACHE["nc"] = {
            "fp32r": _build_module_r,
            "bf16": _build_module_bf16,
            "bf16t": _build_module_bf16_t,
            "bf16s": _build_module_bf16_s,
            "bf16s2": _build_module_bf16_s2,
            "bf16s3": lambda: _build_module_bf16_s(
                tail_sizes=(16, 8, 6, 2), tail_bufs=4
            ),
            "bf16s4": _build_module_bf16_s4,
            "bf16s5": _build_module_bf16_s5,
            "bf16s6": lambda: _build_module_bf16_s(fp8_first=True),
            "bf16s7": _build_module_bf16_s7,
            "bf16s8": lambda: _build_module_bf16_s(bulk_sizes=(48, 48)),
            "bf16sw": lambda: _build_module_bf16_s(warm={84: 10}),
            "bf16sv": lambda: _build_module_bf16_s(
                bulk_sizes=(16, 16, 32, 32), warm={84: 10}, act_alt=True
            ),
            "bf16sa": lambda: _build_module_bf16_s(act_alt=True),
            "bf16sz": lambda: _build_module_bf16_s(store_late=True),
            "bf16sc": lambda: _build_module_bf16_s(
                act_alt=True, store_late=True
            ),
            "fp32": _build_module,
        }[VARIANT]()
    nc = _CACHE["nc"]
    if VARIANT == "bf16s2":
        xf1, xft, bt = _marshal_s2(X, filters, bias)
        in_maps = [
            {"xf1": xf1[k], "xft": xft[k], "bt": bt} for k in range(NCORES)
        ]
    elif VARIANT in ("bf16s4", "bf16s5", "bf16s7"):
        xf1, xf2x, bt = _marshal_s4(X, filters, bias)
        in_maps = [
            {"xf1": xf1[k], "xf2x": xf2x[k], "bt": bt} for k in range(NCORES)
        ]
    elif VARIANT in ("bf16s", "bf16s3", "bf16s6", "bf16s8", "bf16sw", "bf16sv", "bf16sa", "bf16sz", "bf16sc"):
        xf1, xf2x, xft, bt = _marshal_s(X, filters, bias)
        in_maps = [
            {"xf1": xf1[k], "xf2x": xf2x[k], "xft": xft[k], "bt": bt}
            for k in range(NCORES)
        ]
    else:
        mdtype = ml_dtypes.bfloat16 if VARIANT.startswith("bf16") else np.float32
        xfa, bt = _marshal(X, filters, bias, mdtype=mdtype)
        if VARIANT != "fp32":
            bt = np.ascontiguousarray(bt[:FOUT])
        in_maps = [{"xf": xfa[k], "bt": bt} for k in range(NCORES)]
    res = run_bass_kernel_spmd(nc, in_maps, core_ids=list(range(NCORES)))
    LAST_RESULT = res
    outs = [res.results[k]["out"] for k in range(NCORES)]
    if VARIANT == "fp32":
        return _assemble(outs)
    z = _assemble_r(outs)
    return np.ascontiguousarray(z.astype(np.float32)) if z.dtype != np.float32 else z

